# revision 1
# baseline (speedup 1.0000x reference)
"""BetaGNN message-passing kernel for 8 Trainium2 NeuronCores (v2).

Strategy (dest-row sharding, 6250 nodes/core):
  - Host relabels nodes: sorted by in-degree, dealt round-robin to cores so
    every core's tile t has near-identical max-degree -> uniform chunk counts.
  - Hop 1 (AH = A @ relu(x @ W_in^T + b)): no gather. Host pre-gathers the
    3-wide input features per edge (plus a ones column for the bias); the PE
    recomputes h per edge-slot with a K=4 bf16 matmul per 128-edge chunk.
    Edge values (prescaled x16) fold into the relu via per-partition scale on
    ACT/DVE (alternating), output in fp8e4; chunk pairs accumulate into the
    per-tile PSUM with a single fp8 DoubleRow identity matmul (2 K-tiles per
    instruction, constant stationary).
  - 16*AH (fp8e4) is AllGathered across the 8 cores.
  - Hop 2 (A2H = A @ AH): dma_gather of fp8 row PAIRS (512B) from the
    gathered table across 4 SWDGE queues, so int16 indices only need to
    reach 25000. ACT scales the left half, DVE the right half (by 16*val);
    one fp8 DoubleRow identity matmul per chunk accumulates both halves.
  - Dense tail in transposed layout (PE transposes AH/A2H tiles, bf16):
    h2^T = relu((16AH)^T W1/16 + (256A2H)^T W2/256), g = softplus(W_out h2^T
    + b_out); tail blocks are interleaved into hop 2 as their tiles finish.
"""

import sys

for _p in ("/opt/trn_rl_repo", "/root/.axon_site/_ro/trn_rl_repo"):
    if _p not in sys.path:
        sys.path.insert(0, _p)

import numpy as np
import ml_dtypes

import concourse.bacc as bacc
import concourse.bass as bass
import concourse.mybir as mybir
from concourse import tile
from concourse.bass_utils import run_bass_kernel_spmd
from concourse import bass_utils as _bu

# Enable walrus LDWEIGHTS dedup: the identity stationary repeats between
# matmuls; the default =false flag forces a reload per matmul.
_orig_gwa = _bu.get_walrus_args
def _gwa(*a, **k):
    return [str(x).replace("--enable-ldw-opt=false", "--enable-ldw-opt=true")
            for x in _orig_gwa(*a, **k)]
_bu.get_walrus_args = _gwa

F32 = mybir.dt.float32
F32R = mybir.dt.float32r
BF16 = mybir.dt.bfloat16
F8 = mybir.dt.float8e4
I16 = mybir.dt.int16
AF = mybir.ActivationFunctionType
DR = mybir.MatmulPerfMode.DoubleRow
NPF8 = ml_dtypes.float8_e4m3

G_CALL = 16               # chunks (128 idxs each) per dma_gather call
N_QUEUES = 4              # SWDGE queues for gather pipelining


class Cfg:
    def __init__(self, P, E, nc=8, hid=256):
        assert P % (nc * 2) == 0
        self.P, self.E, self.NC, self.HID = P, E, nc, hid
        self.NPC = P // nc                    # nodes per core
        self.NT = (self.NPC + 127) // 128     # dest tiles per core
        self.NPAD = self.NT * 128
        self.BLK = []
        off = 0
        while off < self.NPAD:
            w = min(512, self.NPAD - off)
            self.BLK.append((off, w))
            off += w


def _plan(cfg, deg):
    P, NC, NT = cfg.P, cfg.NC, cfg.NT
    order = np.argsort(-deg, kind="stable")
    rank = np.empty(P, np.int64)
    rank[order] = np.arange(P)
    core_of = rank % NC
    local_of = rank // NC
    gid = core_of * cfg.NPC + local_of
    degs_sorted = deg[order]
    NCHUNK = []
    for t in range(NT):
        NCHUNK.append(max(1, int(degs_sorted[t * 128 * NC])))
    NCHUNK = np.array(NCHUNK, np.int64)
    tile_off = np.concatenate([[0], np.cumsum(NCHUNK)])
    TC = int(tile_off[-1])
    calls = []
    c0 = 0
    while c0 < TC:
        g = min(G_CALL, TC - c0)
        calls.append((c0, g))
        c0 += g
    return core_of, local_of, gid, NCHUNK, tile_off, TC, calls


def _prepare(cfg, beta, degree, A_rows, A_cols, A_vals,
             W_in, b_in, W_mp1, W_mp2, W_out, b_out):
    P, E, NC, NPC = cfg.P, cfg.E, cfg.NC, cfg.NPC
    deg = np.bincount(A_rows, minlength=P).astype(np.int64)
    core_of, local_of, gid, NCHUNK, tile_off, TC, calls = _plan(cfg, deg)
    NSLOT = TC * 128

    d_gid = gid[A_rows.astype(np.int64)]
    oe = np.argsort(d_gid, kind="stable")
    sd = d_gid[oe]
    first = np.r_[True, sd[1:] != sd[:-1]]
    cumstart = np.maximum.accumulate(np.where(first, np.arange(E), 0))
    chunk = np.arange(E) - cumstart
    e_core = sd // NPC
    e_local = sd % NPC
    e_col = e_local % 128
    e_k = tile_off[e_local // 128] + chunk
    e_slot = e_k * 128 + e_col

    src = A_cols.astype(np.int64)[oe]
    vals16 = (A_vals[oe].astype(np.float32)) * 16.0
    sgid = gid[src]
    pidx = (sgid // 2).astype(np.int16)
    half = (sgid % 2).astype(np.int64)

    x4_all = np.stack([beta[:, 0], beta[:, 0] ** 2, degree[:, 0],
                       np.ones(P, np.float32)], axis=0).astype(np.float32)

    NIDXCOL = NSLOT // 16
    per_core = []
    for c in range(NC):
        m = e_core == c
        sl, km, cm, hm = e_slot[m], e_k[m], e_col[m], half[m]
        x4T = np.zeros((4, NSLOT), np.float32)
        x4T[:, sl] = x4_all[:, src[m]]
        # quad-packed layout: chunk 4q+j -> partitions 32j..32j+4, cols q*128
        NQ = (TC + 3) // 4
        x4c = np.zeros((4, NQ * 4, 128), np.float32)
        x4c[:, :TC, :] = x4T.reshape(4, TC, 128)
        x4q = np.zeros((128, NQ * 128), np.float32)
        for j in range(4):
            x4q[32 * j:32 * j + 4, :] = (
                x4c[:, j::4, :].reshape(4, NQ * 128))
        v1 = np.zeros((128, TC), np.float32)
        v1[cm, km] = vals16[m]
        vL = np.zeros((128, TC), np.float32)
        vR = np.zeros((128, TC), np.float32)
        vL[cm[hm == 0], km[hm == 0]] = vals16[m][hm == 0]
        vR[cm[hm == 1], km[hm == 1]] = vals16[m][hm == 1]
        pslot = np.zeros(NSLOT, np.int16)
        pslot[sl] = pidx[m]
        idxh = np.zeros((128, NIDXCOL), np.int16)
        col0 = 0
        soff = 0
        for (c0, g) in calls:
            ni = g * 128
            blockv = pslot[soff:soff + ni].reshape(ni // 16, 16).T
            for q in range(8):
                idxh[16 * q:16 * (q + 1), col0:col0 + ni // 16] = blockv
            col0 += ni // 16
            soff += ni
        per_core.append(dict(
            x4q=x4q.astype(ml_dtypes.bfloat16), v1=v1, vl=vL, vr=vR,
            idx=idxh))

    wiT = np.concatenate([W_in.T.astype(np.float32),
                          b_in[None, :].astype(np.float32)], axis=0)
    wiT4 = np.zeros((128, wiT.shape[1]), np.float32)
    for j in range(4):
        wiT4[32 * j:32 * j + 4, :] = wiT
    idn8p = np.zeros((128, 2, 128), NPF8)
    eye8 = np.eye(128, dtype=np.float32).astype(NPF8)
    idn8p[:, 0, :] = eye8
    idn8p[:, 1, :] = eye8
    consts = dict(
        wit=wiT4.astype(ml_dtypes.bfloat16),
        w1t=np.ascontiguousarray((W_mp1.T / 16.0).astype(ml_dtypes.bfloat16)),
        w2t=np.ascontiguousarray((W_mp2.T / 256.0).astype(ml_dtypes.bfloat16)),
        wot=np.ascontiguousarray(W_out.T.astype(np.float32)),
        bout=np.full((128, 1), float(np.asarray(b_out).reshape(-1)[0]),
                     np.float32),
        idn8p=idn8p,
        idn16=np.eye(128, dtype=np.float32).astype(ml_dtypes.bfloat16),
    )
    meta = dict(NCHUNK=tuple(int(x) for x in NCHUNK), calls=tuple(calls),
                TC=TC, NSLOT=NSLOT, NIDXCOL=NIDXCOL, NQ=(TC + 3) // 4)
    return per_core, consts, meta, (core_of, local_of)


def _build(cfg, meta):
    NT, NPC, NPAD, HID, NC, P = (cfg.NT, cfg.NPC, cfg.NPAD, cfg.HID,
                                 cfg.NC, cfg.P)
    NCHUNK = meta["NCHUNK"]
    calls = meta["calls"]
    TC, NSLOT, NIDXCOL = meta["TC"], meta["NSLOT"], meta["NIDXCOL"]
    tile_off = np.concatenate([[0], np.cumsum(NCHUNK)])
    NBLK = len(cfg.BLK)
    NQ = meta["NQ"]

    nc = bacc.Bacc("TRN2", target_bir_lowering=False, debug=False,
                   num_swdge_queues=N_QUEUES)
    x4T_d = nc.dram_tensor("x4t", [128, NQ * 128], BF16, kind="ExternalInput")
    v1_d = nc.dram_tensor("v1", [128, TC], F32, kind="ExternalInput")
    vl_d = nc.dram_tensor("vl", [128, TC], F32, kind="ExternalInput")
    vr_d = nc.dram_tensor("vr", [128, TC], F32, kind="ExternalInput")
    idx_d = nc.dram_tensor("idx", [128, NIDXCOL], I16, kind="ExternalInput")
    wiT_d = nc.dram_tensor("wit", [128, HID], BF16, kind="ExternalInput")
    w1T_d = nc.dram_tensor("w1t", [HID, HID], BF16, kind="ExternalInput")
    w2T_d = nc.dram_tensor("w2t", [HID, HID], BF16, kind="ExternalInput")
    woT_d = nc.dram_tensor("wot", [HID, 1], F32R, kind="ExternalInput")
    bout_d = nc.dram_tensor("bout", [128, 1], F32, kind="ExternalInput")
    idn8_d = nc.dram_tensor("idn8p", [128, 2 * 128], F8, kind="ExternalInput")
    idn16_d = nc.dram_tensor("idn16", [128, 128], BF16, kind="ExternalInput")
    g_d = nc.dram_tensor("g", [1, NBLK * 512], F32, kind="ExternalOutput")

    ah_bounce = nc.dram_tensor("ah_bounce", [NPC, HID], F8)
    ah_full = nc.dram_tensor("ah_full", [P, HID], F8, addr_space="Shared")

    with tile.TileContext(nc) as tc:
        with (
            tc.tile_pool(name="const", bufs=1) as constp,
            tc.tile_pool(name="xs", bufs=4) as xsp,
            tc.tile_pool(name="msgs", bufs=6) as msgp,
            tc.tile_pool(name="stage", bufs=4) as stagep,
            tc.tile_pool(name="resid", bufs=1) as residp,
            tc.tile_pool(name="pair", bufs=10) as pairp,
            tc.tile_pool(name="ph", bufs=2, space="PSUM") as php,
            tc.tile_pool(name="pz", bufs=2, space="PSUM") as pzp,
            tc.tile_pool(name="pt", bufs=2, space="PSUM") as ptp,
        ):
            wiT = constp.tile([128, HID], BF16, tag="wiT", name="wiT")
            nc.sync.dma_start(wiT[:], wiT_d[:])
            w1T = [constp.tile([128, HID], BF16, tag=f"w1_{k}", name=f"w1_{k}") for k in (0, 1)]
            w2T = [constp.tile([128, HID], BF16, tag=f"w2_{k}", name=f"w2_{k}") for k in (0, 1)]
            for k in (0, 1):
                nc.sync.dma_start(w1T[k][:], w1T_d[128 * k:128 * (k + 1), :])
                nc.sync.dma_start(w2T[k][:], w2T_d[128 * k:128 * (k + 1), :])
            woT = constp.tile([128, 2], F32R, tag="woT", name="woT")
            nc.sync.dma_start(woT[:, 0:1], woT_d[0:128, :])
            nc.sync.dma_start(woT[:, 1:2], woT_d[128:256, :])
            bout = constp.tile([128, 1], F32, tag="bout", name="bout")
            nc.sync.dma_start(bout[:], bout_d[:])
            idn8 = constp.tile([128, 2, 128], F8, tag="idn8", name="idn8")
            nc.sync.dma_start(idn8[:, :, :].rearrange("p a b -> p (a b)"),
                              idn8_d[:])
            idn16 = constp.tile([128, 128], BF16, tag="idn16", name="idn16")
            nc.sync.dma_start(idn16[:], idn16_d[:])
            v1 = constp.tile([128, TC], F32, tag="v1", name="v1")
            nc.sync.dma_start(v1[:], v1_d[:])
            vl = constp.tile([128, TC], F32, tag="vl", name="vl")
            nc.sync.dma_start(vl[:], vl_d[:])
            vr = constp.tile([128, TC], F32, tag="vr", name="vr")
            nc.sync.dma_start(vr[:], vr_d[:])
            idx = constp.tile([128, NIDXCOL], I16, tag="idx", name="idx")
            nc.sync.dma_start(idx[:], idx_d[:])

            ahT = [residp.tile([128, NPAD], BF16, tag=f"ahT{m}", name=f"ahT{m}")
                   for m in (0, 1)]
            a2T = [residp.tile([128, NPAD], BF16, tag=f"a2T{m}", name=f"a2T{m}")
                   for m in (0, 1)]

            # ---- phase A: hop 1 (quad-packed K=4 bf16 matmuls) ----
            t = 0
            pz = None
            pend = []        # m2 pair tiles pending accumulation
            tileends = []
            for g8 in range(0, TC, 8):
                khi = min(g8 + 8, TC)
                xs = xsp.tile([128, 2 * 128], BF16, tag="xs", name="xs")
                q0 = g8 // 4
                hi = min((q0 + 2) * 128, NQ * 128)
                nc.sync.dma_start(xs[:, :hi - q0 * 128],
                                  x4T_d[:, q0 * 128:hi])
                phs = []
                for k in range(g8, khi):
                    j, hf = k % 4, (k - g8) // 4
                    ph = php.tile([128, 512], F32, tag="ph", name="ph",
                                  bufs=4)
                    nc.tensor.matmul(
                        ph[:, :HID],
                        lhsT=xs[32 * j:32 * j + 4,
                                hf * 128:(hf + 1) * 128],
                        rhs=wiT[32 * j:32 * j + 4, :],
                        start=True, stop=True, skip_group_check=True,
                        tile_position=(32 * j, 0))
                    phs.append(ph)
                for k in range(g8, khi):
                    # position within current tile decides pairing
                    if k == int(tile_off[t]):
                        pz = pzp.tile([128, 512], F32, tag="acc", name="acc")
                        pend = []
                    cc = k - int(tile_off[t])
                    if cc % 2 == 0:
                        m2 = msgp.tile([128, 2, HID], F8, tag="m2",
                                       name="m2", bufs=20)
                        pend.append(m2)
                    else:
                        m2 = pend[-1]
                    hslot = cc % 2
                    if k % 2 == 0:
                        nc.scalar.activation(m2[:, hslot, :],
                                             phs[k - g8][:, :HID],
                                             AF.Relu, scale=v1[:, k:k + 1])
                    else:
                        nc.vector.tensor_scalar(
                            m2[:, hslot, :], phs[k - g8][:, :HID],
                            v1[:, k:k + 1], 0.0,
                            op0=mybir.AluOpType.mult,
                            op1=mybir.AluOpType.max)
                    last = (k == int(tile_off[t + 1]) - 1)
                    if cc % 2 == 1:
                        nc.tensor.matmul(
                            pz[:, :HID], lhsT=idn8[:, :, :], rhs=m2[:, :, :],
                            start=(cc == 1), stop=last,
                            perf_mode=DR, skip_group_check=True)
                    elif last:
                        # odd tail chunk: single fp8 matmul
                        nc.tensor.matmul(
                            pz[:, :HID], lhsT=idn8[:, 0, :],
                            rhs=m2[:, 0, :],
                            start=(cc == 0), stop=True,
                            skip_group_check=True)
                    if last:
                        tileends.append((t, pz))
                        t += 1
                for (tt, pzv) in tileends:
                    ah16 = stagep.tile([128, HID], BF16, tag="ah16",
                                       name="ah16")
                    nc.vector.tensor_copy(ah16[:], pzv[:, :HID])
                    ahb = stagep.tile([128, HID], F8, tag="ahb",
                                      name="ahb")
                    nc.scalar.activation(ahb[:], pzv[:, :HID], AF.Copy)
                    rows = min(128, NPC - tt * 128)
                    nc.sync.dma_start(ah_bounce[tt * 128:tt * 128 + rows, :],
                                      ahb[:rows, :])
                    for mh in (0, 1):
                        pt = ptp.tile([128, 512], BF16, tag="pt", name="pt")
                        nc.tensor.transpose(
                            pt[:, :128], ah16[:, mh * 128:(mh + 1) * 128],
                            idn16[:])
                        nc.vector.tensor_copy(
                            ahT[mh][:, tt * 128:(tt + 1) * 128], pt[:, :128])
                tileends = []

            # ---- phase B: allgather of 16*AH (fp8) ----
            nc.gpsimd.collective_compute(
                "AllGather", mybir.AluOpType.bypass,
                replica_groups=[list(range(NC))],
                ins=[ah_bounce.ap().opt()],
                outs=[ah_full.ap().opt()],
            )
            ah_pairs = ah_full.ap().rearrange("(a b) c -> a (b c)", b=2)

            # ---- phase C: hop 2 + interleaved dense tail ----
            def tail_block(b):
                off, w = cfg.BLK[b]
                h2 = []
                for mh in (0, 1):
                    pd = php.tile([128, 512], F32, tag="ph", name="ph",
                                  bufs=4)
                    n = 0
                    for (wt, xt) in ((w1T, ahT), (w2T, a2T)):
                        for k in (0, 1):
                            nc.tensor.matmul(
                                pd[:, :w],
                                lhsT=wt[k][:, mh * 128:(mh + 1) * 128],
                                rhs=xt[k][:, off:off + w],
                                start=(n == 0), stop=(n == 3),
                                skip_group_check=True)
                            n += 1
                    ht = stagep.tile([128, 512], F32R, tag="h2t", name="h2t")
                    nc.scalar.activation(ht[:, :w], pd[:, :w], AF.Relu)
                    h2.append(ht)
                pg = ptp.tile([1, 512], F32, tag="pt", name="pt")
                for k in (0, 1):
                    nc.tensor.matmul(pg[:, :w],
                                     lhsT=woT[:, k:k + 1],
                                     rhs=h2[k][:, :w],
                                     start=(k == 0), stop=(k == 1),
                                     skip_group_check=True)
                gb = stagep.tile([1, 512], F32, tag="gbuf", name="gb",
                                 bufs=4)
                nc.vector.tensor_copy(gb[0:1, :w], pg[:, :w])
                ge = stagep.tile([1, 512], F32, tag="gbuf", name="ge",
                                 bufs=4)
                nc.scalar.activation(ge[0:1, :w], gb[0:1, :w], AF.Exp,
                                     bias=bout[0:1, :])
                go = stagep.tile([1, 512], F32, tag="gbuf", name="go",
                                 bufs=4)
                nc.scalar.activation(go[0:1, :w], ge[0:1, :w], AF.Ln,
                                     bias=1.0)
                nc.sync.dma_start(g_d[0:1, off:off + w], go[0:1, :w])

            t = 0
            pz = pzp.tile([128, 512], F32, tag="acc", name="acc")
            done_in_tile = 0
            for ci, (c0, g) in enumerate(calls):
                ni = g * 128
                col0 = c0 * 8
                pr = pairp.tile([128, G_CALL, 2 * HID], F8,
                                tag="pair", name="pair")
                nc.gpsimd.dma_gather(
                    pr[:, :g, :], ah_pairs,
                    idx[:, col0:col0 + ni // 16],
                    ni, ni, 2 * HID, single_packet=False,
                    queue_num=ci % N_QUEUES)
                for cc in range(g):
                    k = c0 + cc
                    m2 = msgp.tile([128, 2, HID], F8, tag="m2",
                                   name="m2", bufs=20)
                    nc.scalar.activation(m2[:, 0, :], pr[:, cc, 0:HID],
                                         AF.Copy, scale=vl[:, k:k + 1])
                    nc.vector.tensor_scalar_mul(
                        m2[:, 1, :], pr[:, cc, HID:2 * HID],
                        vr[:, k:k + 1])
                    nc.tensor.matmul(
                        pz[:, :HID], lhsT=idn8[:, :, :], rhs=m2[:, :, :],
                        start=(done_in_tile == 0),
                        stop=(done_in_tile == NCHUNK[t] - 1),
                        perf_mode=DR, skip_group_check=True)
                    done_in_tile += 1
                    if done_in_tile == NCHUNK[t]:
                        a2 = stagep.tile([128, HID], BF16, tag="a2",
                                         name="a2")
                        nc.vector.tensor_copy(a2[:], pz[:, :HID])
                        for mh in (0, 1):
                            pt = ptp.tile([128, 512], BF16, tag="pt",
                                          name="pt")
                            nc.tensor.transpose(
                                pt[:, :128], a2[:, mh * 128:(mh + 1) * 128],
                                idn16[:])
                            nc.vector.tensor_copy(
                                a2T[mh][:, t * 128:(t + 1) * 128],
                                pt[:, :128])
                        if t % 4 == 3:
                            tail_block(t // 4)
                        t += 1
                        done_in_tile = 0
                        if t < NT:
                            pz = pzp.tile([128, 512], F32, tag="acc",
                                          name="acc")
            # leftover tail blocks (tiles past the last %4==3 boundary)
            for b in range(NT // 4, NBLK):
                tail_block(b)

    nc.compile()
    return nc


_COMPILED = {}


def _get_compiled(cfg, meta):
    key = (cfg.P, cfg.E, meta["NCHUNK"], meta["calls"])
    if key not in _COMPILED:
        _COMPILED[key] = _build(cfg, meta)
    return _COMPILED[key]


def run(cfg, inputs, trace=False):
    per_core, consts, meta, (core_of, local_of) = _prepare(cfg, **inputs)
    ncobj = _get_compiled(cfg, meta)
    in_maps = []
    for c in range(cfg.NC):
        pc = per_core[c]
        im = {"x4t": pc["x4q"], "v1": pc["v1"], "vl": pc["vl"],
              "vr": pc["vr"], "idx": pc["idx"]}
        im.update({k: np.asarray(v) for k, v in consts.items()})
        im["idn8p"] = im["idn8p"].reshape(128, 256)
        in_maps.append(im)
    res = run_bass_kernel_spmd(ncobj, in_maps, list(range(cfg.NC)),
                               trace=trace)
    g = np.empty(cfg.P, np.float32)
    for c in range(cfg.NC):
        go = np.asarray(res.results[c]["g"]).reshape(-1)
        mine = core_of == c
        g[mine] = go[local_of[mine]]
    return g.reshape(cfg.P, 1), res


def kernel(**inputs):
    cfg = Cfg(P=50000, E=800000)
    g, _ = run(cfg, inputs)
    return g

